# revision 41
# baseline (speedup 1.0000x reference)
"""GRU (hard-sigmoid gates, tanh candidate) Trainium2 kernel, 8 NeuronCores.

Strategy:
  - Data-parallel: batch 32 -> 4 per core. The T=512 recurrence is strictly
    sequential, and collectives have a ~5us floor, so each core runs its own
    batch shard's full recurrence locally (no cross-core traffic).
  - Everything lives transposed: h^T [U_part, B_free], mx^T [3U_part, T, B].
    The recurrent matmul uses the native recurrent_kernel [U, 3U] layout as
    the stationary operand (lhsT), streaming h^T [128, 4] as the moving
    operand -> output lands already transposed, elementwise ops use all 128
    partitions, and no per-step transposes are needed.
  - The per-step cost is LDWEIGHTS-bound: 48 stationary loads of [128,128]
    stream the whole recurrent weight matrix into the PE each step. The
    recurrent weights are stored as fp8 E3M4 (float8e3): FWL (fast weight
    load, compiler-automatic for 128-col non-fp32 stationaries) reads 4
    bytes/cycle, so an fp8 load is ~2x faster than bf16. Moving operands
    (h, rh, mx) stay bf16; the PE upconverts both sides to FP22 internally.
  - fp8 range handling: Wr_zr is stored as e3m4(25.6*Wr) (the hard-sigmoid
    0.2 and a x128 gate scale folded in); Wr_h as e3m4(32*Wr). The input
    projection kernel is folded x25.6 (z/r cols, bias' = 128*(0.2b+0.5))
    and x4096 (hh cols), so PSUM accumulates 128*hardsig_arg for z/r and
    4096*tanh_arg for hh. Descale is free: z32=clip(psum,0,128) IS 128*z
    (DVE 2-op clip), w = 1-z = tensor_scalar(z32, -1/128, +1), rh' = z-like
    r32*h = 128*(r*h) feeds the hh matmul whose x32 weights make the scales
    meet at 4096; tanh reads PSUM with Act scale=1/4096.
  - One PSUM tile [128, 12, 1, B] per step (own bank, ping-pong bufs=2)
    holds all gates; a single identity matmul (N=48) initializes it with mx
    (start=True), then 48 weight MMs accumulate into slices. This replaces
    the previous 12 id-MMs.
  - Blend without descale ops: w = 1-z, e = w*h, he = h-e (= z*h, both off
    the critical path), then per half: hh = tanh, f = w*hh, h' = he + f.
  - h' is written directly into a persistent bf16 history buffer (slot s+1);
    the next step's matmuls read slot s. History bulk-DMAs to DRAM (bf16)
    every 64 steps; host upcasts to f32.
  - Outer For_i hardware loop (4 iters) x 64 python-unrolled steps with
    ping-pong prefetch of the staged mx blocks.
"""

import os
import sys
from contextlib import ExitStack

sys.path.insert(0, "/opt/trn_rl_repo")

import numpy as np
import ml_dtypes

import concourse.bass as bass
import concourse.tile as tile
from concourse import bacc, mybir
from concourse.bass_utils import run_bass_kernel_spmd
from concourse.masks import make_identity
from concourse.tile_autobufs import add_dep_helper


def _install_ntff_hook():
    """The container's antenv stub lacks axon_hooks; provide it so
    trace=True (used by test.py for profiling) works. No-op on failure."""
    import types

    try:
        import antenv
        if "antenv.axon_hooks" in sys.modules:
            return
        mod = types.ModuleType("antenv.axon_hooks")
        state = {"h": None}
        mod.set_axon_ntff_profile_hook = lambda h: state.__setitem__("h", h)
        mod.get_axon_ntff_profile_hook = lambda: state["h"]
        sys.modules["antenv.axon_hooks"] = mod
        antenv.axon_hooks = mod
        from trn_agent_boot.trn_boot import _ntff_profile_via_ctypes
        mod.set_axon_ntff_profile_hook(
            _ntff_profile_via_ctypes("/opt/axon/libaxon_pjrt.so")
        )
    except Exception:
        pass


_install_ntff_hook()

B, T, D, U = 32, 512, 512, 512
NCORES = 8
BL = B // NCORES          # 4 batches per core
KC = D // 128             # 4 contraction chunks (input proj)
UC = U // 128             # 4 contraction chunks (recurrent)
M_ALL = 3 * U // 128      # 12 output column chunks
SBLK = 64                 # steps per staged mx block
BODY = 2 * SBLK           # steps per For_i body (ping-pong A/B)

# gate scales (see module docstring)
S_GATE = 128.0            # z/r psum = S_GATE * hard_sigmoid_arg
S_HH = 4096.0             # hh psum = S_HH * tanh_arg
S_WZR = S_GATE * 0.2      # fold of 0.2 and gate scale into z/r weights
S_WH = S_HH / S_GATE      # hh weight scale; rh' carries S_GATE

BF16 = mybir.dt.bfloat16
FP8 = mybir.dt.float8e3
F32 = mybir.dt.float32
Alu = mybir.AluOpType
Act = mybir.ActivationFunctionType
ET = mybir.EngineType

_CACHE = {}
LAST_RESULT = None


def _build(T=T):
    nc = bacc.Bacc()
    xT = nc.declare_dram_parameter("xT", [D, BL * T], BF16, isOutput=False)
    wk = nc.declare_dram_parameter("wk", [D, 3 * U], BF16, isOutput=False)
    wr = nc.declare_dram_parameter("wr", [U, 3 * U], FP8, isOutput=False)
    bp = nc.declare_dram_parameter("bp", [3 * U], F32, isOutput=False)
    # out[u%128, u//128, t, b] (bf16; host upcasts)
    out = nc.declare_dram_parameter("out", [128, UC, T, BL], BF16, isOutput=True)

    with tile.TileContext(nc) as tc, ExitStack() as ctx:
        consts = ctx.enter_context(tc.tile_pool(name="consts", bufs=1))
        psum_p = ctx.enter_context(tc.tile_pool(name="psum", bufs=2, space="PSUM"))
        psum_1 = ctx.enter_context(tc.tile_pool(name="psum1", bufs=1, space="PSUM"))
        psum_z = ctx.enter_context(tc.tile_pool(name="psumz", bufs=2, space="PSUM"))
        work = ctx.enter_context(tc.tile_pool(name="work", bufs=2))

        wk_sb = consts.tile([128, KC, 3 * U], BF16)
        nc.sync.dma_start(out=wk_sb, in_=wk.rearrange("(c p) n -> p c n", p=128))
        wr_sb = consts.tile([128, UC, 3 * U], FP8)
        nc.sync.dma_start(out=wr_sb, in_=wr.rearrange("(c p) n -> p c n", p=128))
        bp_sb = consts.tile([128, M_ALL], F32)
        nc.sync.dma_start(out=bp_sb, in_=bp.rearrange("(m p) -> p m", p=128))
        # xT is t-major [D, T*BL] host-side; load it in t-blocks so phase-1
        # t-block 0 starts after 1/4 of x arrives instead of the whole tensor
        xT_sb = consts.tile([128, KC, T * BL], BF16)
        xT_r = xT.rearrange("(c p) n -> p c n", p=128)
        TBLK = 128 * BL
        for tb in range(T // 128):
            nc.sync.dma_start(out=xT_sb[:, :, tb * TBLK:(tb + 1) * TBLK],
                              in_=xT_r[:, :, tb * TBLK:(tb + 1) * TBLK])
        ident = consts.tile([128, 128], FP8)
        make_identity(nc, ident)

        # mx^T [n%128, t, n//128, b] bf16 (t-major so one step's mx for all
        # gates is a contiguous 48-element row -> fast id-MM moving reads),
        # padded by BODY junk steps so the ping-pong prefetch can always
        # read a full block
        mx_sb = consts.tile([128, T + BODY, M_ALL, BL], BF16)
        nc.vector.memset(mx_sb[:, T:, :, :], 0.0)

        # ---- phase 1: mx^T = kernel^T @ x^T (+ bias', scales pre-folded) ----
        # The PE queue is in-order, so phase-1 matmuls emitted before the
        # recurrence would serialize with it. Only t-block 0 is emitted up
        # front; blocks 1..3 are emitted *inside* the first peeled steps
        # (see below) so they fill the PE's idle windows during the
        # recurrence's DVE/Act phases.
        xT_bt = xT_sb.rearrange("p c (t b) -> p c t b", b=BL)
        TB = T // 128

        def p1_pair(tb, m):
            ps = psum_p.tile([128, BL * 128], F32, tag="p1")
            for d in range(KC):
                nc.tensor.matmul(
                    ps,
                    lhsT=wk_sb[:, d, m * 128:(m + 1) * 128],
                    rhs=xT_bt[:, d, tb * 128:(tb + 1) * 128, :],
                    start=(d == 0),
                    stop=(d == KC - 1),
                )
            # psum free order is (t, b), matching the t-major mx layout
            nc.scalar.activation(
                out=mx_sb[:, tb * 128:(tb + 1) * 128, m, :],
                in_=ps, func=Act.Identity,
                bias=bp_sb[:, m:m + 1],
            )

        for m in range(M_ALL):
            p1_pair(0, m)
        p1_rest = [(tb, m) for tb in range(1, TB) for m in range(M_ALL)]

        # ---- phase 2: recurrence ----
        # persistent bf16 history: step s reads slot s, writes slot s+1;
        # the last step wraps to slot 0 (becomes next body's h_in) so no
        # carry copy is needed.
        hist = consts.tile([128, UC, BODY, BL], BF16)
        nc.vector.memset(hist[:, :, 0:1, :], 0.0)
        stgA = consts.tile([128, SBLK, M_ALL, BL], BF16)
        stgB = consts.tile([128, SBLK, M_ALL, BL], BF16)
        nc.sync.dma_start(out=stgA, in_=mx_sb[:, 0:SBLK, :, :])

        # PSUM mx-init. Block tops use one consolidated identity matmul per
        # tile (start=True): it both writes mx and primes the bank's
        # has_written bits so later matmuls accumulate. All other steps
        # initialize via Act-engine Copy writes into PSUM (the Act engine is
        # nearly idle): the PE's has_written bits survive (only a start=True
        # clears them), so the start=False weight MMs accumulate on top of
        # the Act-written mx with no PE instructions spent at all.
        # PSUM buffering: pr/pzz are single-buffered (their WAR partners rc/
        # z32 fire early in the cycle, so next-step ids never stall on them);
        # phA/phB are double-buffered so their ids don't wait on this step's
        # tanh reads. Banks: p1(2) + r(1) + z(1) + hhA(2) + hhB(2) = 8.
        def alloc_rz(stg, s, ids):
            pr = psum_p.tile([128, 4, 1, BL], F32, tag="r", name="pr", bufs=1)
            pzz = psum_z.tile([128, 4, 1, BL], F32, tag="z", name="pzz",
                              bufs=1)
            if ids:
                nc.tensor.matmul(pr[:, :, 0, :], lhsT=ident,
                                 rhs=stg[:, s, 4:8, :],
                                 start=True, stop=False, skip_group_check=True)
                nc.tensor.matmul(pzz[:, :, 0, :], lhsT=ident,
                                 rhs=stg[:, s, 0:4, :],
                                 start=True, stop=False, skip_group_check=True)
            else:
                nc.scalar.activation(out=pr[:, :, 0, :], in_=stg[:, s, 4:8, :],
                                     func=Act.Copy)
                nc.scalar.activation(out=pzz[:, :, 0, :],
                                     in_=stg[:, s, 0:4, :], func=Act.Copy)
            return pr, pzz

        def alloc_hh(stg, s, ids):
            phA = psum_1.tile([128, 2, 1, BL], F32, tag="hhA", name="phA",
                              bufs=2)
            phB = psum_1.tile([128, 2, 1, BL], F32, tag="hhB", name="phB",
                              bufs=2)
            if ids:
                nc.tensor.matmul(phA[:, :, 0, :], lhsT=ident,
                                 rhs=stg[:, s, 8:10, :],
                                 start=True, stop=False, skip_group_check=True)
                nc.tensor.matmul(phB[:, :, 0, :], lhsT=ident,
                                 rhs=stg[:, s, 10:12, :],
                                 start=True, stop=False, skip_group_check=True)
            else:
                nc.scalar.activation(out=phA[:, :, 0, :],
                                     in_=stg[:, s, 8:10, :], func=Act.Copy)
                nc.scalar.activation(out=phB[:, :, 0, :],
                                     in_=stg[:, s, 10:12, :], func=Act.Copy)
            return phA, phB

        def step(stg, s, slot, tiles, nxt, next_ids=False, extra=None):
            out_slot = (slot + 1) % BODY
            h_in = hist[:, :, slot, :]                    # [128, UC, BL] bf16
            h_in4 = hist[:, :, slot:slot + 1, :]          # [128, UC, 1, BL]
            pr, pzz, phA, phB = tiles
            # r-gate weight MMs first, k-outer so the k=0,1 MMs only need the
            # first half of the blended h (chunked handoff from prev step)
            for k in range(UC):
                for m in range(4):
                    r_i = nc.tensor.matmul(
                        pr[:, m, 0, :],
                        lhsT=wr_sb[:, k, (4 + m) * 128:(5 + m) * 128],
                        rhs=h_in[:, k, :],
                        start=False,
                        stop=(k == UC - 1 and m == 3),
                        skip_group_check=True,
                    )
            for k in range(UC):
                for m in range(4):
                    z_i = nc.tensor.matmul(
                        pzz[:, m, 0, :],
                        lhsT=wr_sb[:, k, m * 128:(m + 1) * 128],
                        rhs=h_in[:, k, :],
                        start=False,
                        stop=(k == UC - 1 and m == 3),
                        skip_group_check=True,
                    )
                    add_dep_helper(z_i.ins, r_i.ins, sync=False,
                                   reason="keep all r MMs before z MMs")

            # r32 = clip(psum_r, 0, 128) = 128*r;  rh' = r32*h = 128*(r*h)
            # (unblocks hh matmuls)
            r_bf = work.tile([128, 4, 1, BL], BF16, tag="rbf")
            nc.vector.tensor_scalar(r_bf, pr, S_GATE, 0.0,
                                    op0=Alu.min, op1=Alu.max)
            # rh in halves: the hh k=0,1 matmuls wait only on rh chunks 0,1
            # (subtile deps), starting ~110ns earlier than a full-rh op
            rh = work.tile([128, UC, 1, BL], BF16, tag="rh")
            rha_i = nc.vector.tensor_mul(rh[:, 0:2, :, :], r_bf[:, 0:2, :, :],
                                         h_in4[:, 0:2, :, :])
            rh_i = nc.vector.tensor_mul(rh[:, 2:4, :, :], r_bf[:, 2:4, :, :],
                                        h_in4[:, 2:4, :, :])
            add_dep_helper(rh_i.ins, rha_i.ins, sync=False,
                           reason="rh chunks 01 first for early hh start")
            # hh pre-activation: psum = mx_h' + rh' @ W_h'; m-halves so
            # tanh_A can run while the second-half matmuls still execute
            for m in range(2):
                for k in range(UC):
                    nc.tensor.matmul(
                        phA[:, m, 0, :],
                        lhsT=wr_sb[:, k, 2 * U + m * 128:2 * U + (m + 1) * 128],
                        rhs=rh[:, k, 0, :],
                        start=False,
                        stop=(m == 1 and k == UC - 1),
                        skip_group_check=True,
                    )
            for m in range(2, 4):
                for k in range(UC):
                    nc.tensor.matmul(
                        phB[:, m - 2, 0, :],
                        lhsT=wr_sb[:, k, 2 * U + m * 128:2 * U + (m + 1) * 128],
                        rhs=rh[:, k, 0, :],
                        start=False,
                        stop=(m == 3 and k == UC - 1),
                        skip_group_check=True,
                    )
            # z ops off the critical chain (clip_z ordered after rh):
            # z32 = clip(psum_z, 0, 128) = 128*z; he = z*h = (z32*h)/128
            # (exact: /128 is a power of two) -- he does NOT depend on w, so
            # it clears the DVE queue before the post-tanh f/add tail;
            # w = 1 - z32/128 is only needed by f.
            z32 = work.tile([128, 4, 1, BL], F32, tag="z32")
            zb_i = nc.vector.tensor_scalar(z32, pzz, S_GATE, 0.0,
                                           op0=Alu.min, op1=Alu.max)
            add_dep_helper(zb_i.ins, rh_i.ins, sync=False,
                           reason="DVE critical chain first")
            a32 = work.tile([128, 4, 1, BL], F32, tag="a32")
            nc.vector.tensor_mul(a32, z32, h_in4)                   # 128*z*h
            he_t = work.tile([128, 4, 1, BL], F32, tag="het")
            he_i = nc.vector.tensor_scalar(he_t, a32, 1.0 / S_GATE, 0.0,
                                           op0=Alu.mult, op1=Alu.bypass)
            w_t = work.tile([128, 4, 1, BL], F32, tag="wt")
            nc.vector.tensor_scalar(w_t, z32, -1.0 / S_GATE, 1.0,
                                    op0=Alu.mult, op1=Alu.add)      # 1-z
            # hh = tanh(psum/S_HH); h' = he + w*hh, in halves -> hist out_slot
            hh_A = work.tile([128, 2, 1, BL], F32, tag="hhA2")
            nc.scalar.activation(out=hh_A, in_=phA,
                                 func=Act.Tanh, scale=1.0 / S_HH)
            f_A = work.tile([128, 2, 1, BL], F32, tag="ftA")
            fa_i = nc.vector.tensor_mul(f_A, w_t[:, 0:2, :, :], hh_A)
            add_dep_helper(fa_i.ins, he_i.ins, sync=False,
                           reason="he off-critical, keep before f_A")
            adda_i = nc.vector.tensor_add(
                hist[:, 0:2, out_slot:out_slot + 1, :],
                f_A, he_t[:, 0:2, :, :])
            hh_B = work.tile([128, 2, 1, BL], F32, tag="hhB2")
            nc.scalar.activation(out=hh_B, in_=phB,
                                 func=Act.Tanh, scale=1.0 / S_HH)
            f_B = work.tile([128, 2, 1, BL], F32, tag="ftB")
            fb_i = nc.vector.tensor_mul(f_B, w_t[:, 2:4, :, :], hh_B)
            add_dep_helper(fb_i.ins, adda_i.ins, sync=True,
                           reason="add_A gates next r MMs, keep before f_B")
            nc.vector.tensor_add(hist[:, 2:4, out_slot:out_slot + 1, :],
                                 f_B, he_t[:, 2:4, :, :])
            # next step's PSUM mx-init ids, emitted at the tail so they run
            # back-to-back in the blend idle window: phA/phB first (bufs=2,
            # wait-free), then pr/pzz (their WAR partners rc/z32 of THIS
            # step have already fired by then)
            tiles_next = None
            if extra is not None:
                p1_pair(*extra)
            if nxt is not None:
                tiles_next = alloc_rz(*nxt, ids=next_ids) \
                    + alloc_hh(*nxt, ids=next_ids)
            return tiles_next

        # Run a 64-step sub-block: the first step's PSUM init at the block
        # top (one id group per 64 steps primes values + has_written bits),
        # every later step's init one-ahead in the previous step's tail via
        # Act copies. Keeping the carry inside the sub-block avoids
        # loop-carried psum tiles across the For_i back-edge (which
        # deadlocks the tile scheduler). prime_two=True also uses id-MMs for
        # step 1 (the very first block: that buffer set's has_written bits
        # have never been set by any matmul yet).
        def run_block(stg, base_slot, extras=None, prime_two=False):
            tiles = alloc_rz(stg, 0, ids=True) + alloc_hh(stg, 0, ids=True)
            for s in range(SBLK):
                nxt = (stg, s + 1) if s < SBLK - 1 else None
                extra = extras.get(s) if extras else None
                tiles = step(stg, s, base_slot + s, tiles, nxt,
                             next_ids=True, extra=extra)

        # ---- peeled first block (t = 0..BODY-1), python-level so the
        # remaining phase-1 t-blocks can be interleaved into its steps,
        # one piece every 3rd step (deadlines: tb1 before the stgA refill
        # at step 64, tb2/tb3 before the For_i prefetches need them) ----
        p1_sched = {2 * i: p1_rest[i] for i in range(len(p1_rest))}
        nc.sync.dma_start(out=stgB, in_=mx_sb[:, SBLK:2 * SBLK, :, :])
        run_block(stgA, 0, extras={s: p for s, p in p1_sched.items()
                                   if s < SBLK}, prime_two=True)
        nc.sync.dma_start(out=stgA, in_=mx_sb[:, BODY:BODY + SBLK, :, :])
        run_block(stgB, SBLK, extras={s - SBLK: p for s, p in p1_sched.items()
                                      if s >= SBLK})
        nc.sync.dma_start(out=out[:, :, 0:BODY - 1, :],
                          in_=hist[:, :, 1:BODY, :])
        nc.sync.dma_start(out=out[:, :, BODY - 1:BODY, :],
                          in_=hist[:, :, 0:1, :])

        with tc.For_i(BODY, T, BODY, staggered_reset=True,
                      hint_engines=(ET.PE, ET.DVE, ET.Activation,
                                    ET.SP, ET.Pool)) as i:
            nc.sync.dma_start(out=stgB,
                              in_=mx_sb[:, bass.ds(i + SBLK, SBLK), :, :])
            run_block(stgA, 0)
            nc.sync.dma_start(out=stgA,
                              in_=mx_sb[:, bass.ds(i + BODY, SBLK), :, :])
            run_block(stgB, SBLK)
            nc.sync.dma_start(out=out[:, :, bass.ds(i, BODY - 1), :],
                              in_=hist[:, :, 1:BODY, :])
            nc.sync.dma_start(out=out[:, :, bass.ds(i + BODY - 1, 1), :],
                              in_=hist[:, :, 0:1, :])
    return nc


def _graph():
    if "nc" not in _CACHE:
        nc = _build()
        if not nc.is_finalized():
            nc.finalize()
        _CACHE["nc"] = nc
    return _CACHE["nc"]


def kernel(x, kernel, recurrent_kernel, bias):
    global LAST_RESULT
    x = np.asarray(x, dtype=np.float32)
    wk_f = np.asarray(kernel, dtype=np.float32)
    wr_f = np.asarray(recurrent_kernel, dtype=np.float32)
    b_f = np.asarray(bias, dtype=np.float32)

    # fold the hard_sigmoid 0.2 and the fp8/gate scales into weights+bias
    scale = np.full((3 * U,), S_WH, np.float32)
    scale[: 2 * U] = S_WZR
    wk_scale = np.full((3 * U,), S_HH, np.float32)
    wk_scale[: 2 * U] = S_WZR
    wk_h = (wk_f * wk_scale).astype(ml_dtypes.bfloat16)
    wr_h = np.clip(wr_f * scale, -15.0, 15.0).astype(ml_dtypes.float8_e3m4)
    bp_h = np.where(np.arange(3 * U) < 2 * U,
                    S_GATE * (0.2 * b_f + 0.5), S_HH * b_f).astype(np.float32)

    in_maps = []
    for c in range(NCORES):
        xs = x[c * BL:(c + 1) * BL]                       # [BL, T, D]
        xTc = np.ascontiguousarray(
            xs.transpose(2, 1, 0).reshape(D, T * BL)      # t-major
        ).astype(ml_dtypes.bfloat16)
        in_maps.append({"xT": xTc, "wk": wk_h, "wr": wr_h, "bp": bp_h})

    res = run_bass_kernel_spmd(
        _graph(), in_maps, core_ids=list(range(NCORES)),
        trace=bool(os.environ.get("GRU_TRACE")),
    )
    LAST_RESULT = res

    outs = []
    for c in range(NCORES):
        arr = np.asarray(res.results[c]["out"]).astype(np.float32)
        outs.append(np.transpose(arr, (3, 2, 1, 0)).reshape(BL, T, U))
    return np.concatenate(outs, axis=0)


# revision 43
# speedup vs baseline: 1.0868x; 1.0868x over previous
"""GRU (hard-sigmoid gates, tanh candidate) Trainium2 kernel, 8 NeuronCores.

Strategy:
  - Data-parallel: batch 32 -> 4 per core. The T=512 recurrence is strictly
    sequential, and collectives have a ~5us floor, so each core runs its own
    batch shard's full recurrence locally (no cross-core traffic).
  - Everything lives transposed: h^T [U_part, B_free], mx^T [3U_part, T, B].
    The recurrent matmul uses the native recurrent_kernel [U, 3U] layout as
    the stationary operand (lhsT), streaming h^T [128, 4] as the moving
    operand -> output lands already transposed, elementwise ops use all 128
    partitions, and no per-step transposes are needed.
  - The per-step cost is LDWEIGHTS-bound: 48 stationary loads of [128,128]
    stream the whole recurrent weight matrix into the PE each step. The
    recurrent weights are stored as fp8 E3M4 (float8e3): FWL (fast weight
    load, compiler-automatic for 128-col non-fp32 stationaries) reads 4
    bytes/cycle, so an fp8 load is ~2x faster than bf16. Moving operands
    (h, rh, mx) stay bf16; the PE upconverts both sides to FP22 internally.
  - fp8 range handling: Wr_zr is stored as e3m4(25.6*Wr) (the hard-sigmoid
    0.2 and a x128 gate scale folded in); Wr_h as e3m4(32*Wr). The input
    projection kernel is folded x25.6 (z/r cols, bias' = 128*(0.2b+0.5))
    and x4096 (hh cols), so PSUM accumulates 128*hardsig_arg for z/r and
    4096*tanh_arg for hh. Descale is free: z32=clip(psum,0,128) IS 128*z
    (DVE 2-op clip), w = 1-z = tensor_scalar(z32, -1/128, +1), rh' = z-like
    r32*h = 128*(r*h) feeds the hh matmul whose x32 weights make the scales
    meet at 4096; tanh reads PSUM with Act scale=1/4096.
  - One PSUM tile [128, 12, 1, B] per step (own bank, ping-pong bufs=2)
    holds all gates; a single identity matmul (N=48) initializes it with mx
    (start=True), then 48 weight MMs accumulate into slices. This replaces
    the previous 12 id-MMs.
  - Blend without descale ops: w = 1-z, e = w*h, he = h-e (= z*h, both off
    the critical path), then per half: hh = tanh, f = w*hh, h' = he + f.
  - h' is written directly into a persistent bf16 history buffer (slot s+1);
    the next step's matmuls read slot s. History bulk-DMAs to DRAM (bf16)
    every 64 steps; host upcasts to f32.
  - Outer For_i hardware loop (4 iters) x 64 python-unrolled steps with
    ping-pong prefetch of the staged mx blocks.
"""

import os
import sys
from contextlib import ExitStack

sys.path.insert(0, "/opt/trn_rl_repo")

import numpy as np
import ml_dtypes

import concourse.bass as bass
import concourse.tile as tile
from concourse import bacc, mybir
from concourse.bass_utils import run_bass_kernel_spmd
from concourse.masks import make_identity
from concourse.tile_autobufs import add_dep_helper


def _install_ntff_hook():
    """The container's antenv stub lacks axon_hooks; provide it so
    trace=True (used by test.py for profiling) works. No-op on failure."""
    import types

    try:
        import antenv
        if "antenv.axon_hooks" in sys.modules:
            return
        mod = types.ModuleType("antenv.axon_hooks")
        state = {"h": None}
        mod.set_axon_ntff_profile_hook = lambda h: state.__setitem__("h", h)
        mod.get_axon_ntff_profile_hook = lambda: state["h"]
        sys.modules["antenv.axon_hooks"] = mod
        antenv.axon_hooks = mod
        from trn_agent_boot.trn_boot import _ntff_profile_via_ctypes
        mod.set_axon_ntff_profile_hook(
            _ntff_profile_via_ctypes("/opt/axon/libaxon_pjrt.so")
        )
    except Exception:
        pass


_install_ntff_hook()

B, T, D, U = 32, 512, 512, 512
NCORES = 8
BL = B // NCORES          # 4 batches per core
KC = D // 128             # 4 contraction chunks (input proj)
UC = U // 128             # 4 contraction chunks (recurrent)
M_ALL = 3 * U // 128      # 12 output column chunks
SBLK = 64                 # steps per staged mx block
BODY = 2 * SBLK           # steps per For_i body (ping-pong A/B)

# gate scales (see module docstring)
S_GATE = 128.0            # z/r psum = S_GATE * hard_sigmoid_arg
S_HH = 4096.0             # hh psum = S_HH * tanh_arg
S_WZR = S_GATE * 0.2      # fold of 0.2 and gate scale into z/r weights
S_WH = S_HH / S_GATE      # hh weight scale; rh' carries S_GATE

BF16 = mybir.dt.bfloat16
FP8 = mybir.dt.float8e3
F32 = mybir.dt.float32
Alu = mybir.AluOpType
Act = mybir.ActivationFunctionType
ET = mybir.EngineType

_CACHE = {}
LAST_RESULT = None


def _build(T=T):
    nc = bacc.Bacc()
    xT = nc.declare_dram_parameter("xT", [D, BL * T], BF16, isOutput=False)
    wk = nc.declare_dram_parameter("wk", [D, 3 * U], BF16, isOutput=False)
    wr = nc.declare_dram_parameter("wr", [U, 3 * U], FP8, isOutput=False)
    bp = nc.declare_dram_parameter("bp", [3 * U], F32, isOutput=False)
    # out[u%128, u//128, t, b] (bf16; host upcasts)
    out = nc.declare_dram_parameter("out", [128, UC, T, BL], BF16, isOutput=True)

    with tile.TileContext(nc) as tc, ExitStack() as ctx:
        consts = ctx.enter_context(tc.tile_pool(name="consts", bufs=1))
        psum_p = ctx.enter_context(tc.tile_pool(name="psum", bufs=2, space="PSUM"))
        psum_1 = ctx.enter_context(tc.tile_pool(name="psum1", bufs=1, space="PSUM"))
        psum_z = ctx.enter_context(tc.tile_pool(name="psumz", bufs=2, space="PSUM"))
        work = ctx.enter_context(tc.tile_pool(name="work", bufs=2))

        wk_sb = consts.tile([128, KC, 3 * U], BF16)
        nc.sync.dma_start(out=wk_sb, in_=wk.rearrange("(c p) n -> p c n", p=128))
        wr_sb = consts.tile([128, UC, 3 * U], FP8)
        nc.sync.dma_start(out=wr_sb, in_=wr.rearrange("(c p) n -> p c n", p=128))
        bp_sb = consts.tile([128, M_ALL], F32)
        nc.sync.dma_start(out=bp_sb, in_=bp.rearrange("(m p) -> p m", p=128))
        # xT is t-major [D, T*BL] host-side; load it in t-blocks so phase-1
        # t-block 0 starts after 1/4 of x arrives instead of the whole tensor
        xT_sb = consts.tile([128, KC, T * BL], BF16)
        xT_r = xT.rearrange("(c p) n -> p c n", p=128)
        TBLK = 128 * BL
        for tb in range(T // 128):
            nc.sync.dma_start(out=xT_sb[:, :, tb * TBLK:(tb + 1) * TBLK],
                              in_=xT_r[:, :, tb * TBLK:(tb + 1) * TBLK])
        ident = consts.tile([128, 128], FP8)
        make_identity(nc, ident)

        # mx^T [n%128, t, n//128, b] bf16 (t-major so one step's mx for all
        # gates is a contiguous 48-element row -> fast id-MM moving reads),
        # padded by BODY junk steps so the ping-pong prefetch can always
        # read a full block
        mx_sb = consts.tile([128, T + BODY, M_ALL, BL], BF16)
        nc.vector.memset(mx_sb[:, T:, :, :], 0.0)

        # ---- phase 1: mx^T = kernel^T @ x^T (+ bias', scales pre-folded) ----
        # The PE queue is in-order, so phase-1 matmuls emitted before the
        # recurrence would serialize with it. Only t-block 0 is emitted up
        # front; blocks 1..3 are emitted *inside* the first peeled steps
        # (see below) so they fill the PE's idle windows during the
        # recurrence's DVE/Act phases.
        xT_bt = xT_sb.rearrange("p c (t b) -> p c t b", b=BL)
        TB = T // 128

        def p1_pair(tb, m):
            ps = psum_p.tile([128, BL * 128], F32, tag="p1")
            for d in range(KC):
                nc.tensor.matmul(
                    ps,
                    lhsT=wk_sb[:, d, m * 128:(m + 1) * 128],
                    rhs=xT_bt[:, d, tb * 128:(tb + 1) * 128, :],
                    start=(d == 0),
                    stop=(d == KC - 1),
                )
            # psum free order is (t, b), matching the t-major mx layout
            nc.scalar.activation(
                out=mx_sb[:, tb * 128:(tb + 1) * 128, m, :],
                in_=ps, func=Act.Identity,
                bias=bp_sb[:, m:m + 1],
            )

        for m in range(M_ALL):
            p1_pair(0, m)
        p1_rest = [(tb, m) for tb in range(1, TB) for m in range(M_ALL)]

        # ---- phase 2: recurrence ----
        # persistent bf16 history: step s reads slot s, writes slot s+1;
        # the last step wraps to slot 0 (becomes next body's h_in) so no
        # carry copy is needed.
        hist = consts.tile([128, UC, BODY, BL], BF16)
        nc.vector.memset(hist[:, :, 0:1, :], 0.0)
        stgA = consts.tile([128, SBLK, M_ALL, BL], BF16)
        stgB = consts.tile([128, SBLK, M_ALL, BL], BF16)
        nc.sync.dma_start(out=stgA, in_=mx_sb[:, 0:SBLK, :, :])

        # PSUM mx-init. Block tops use one consolidated identity matmul per
        # tile (start=True): it both writes mx and primes the bank's
        # has_written bits so later matmuls accumulate. All other steps
        # initialize via Act-engine Copy writes into PSUM (the Act engine is
        # nearly idle): the PE's has_written bits survive (only a start=True
        # clears them), so the start=False weight MMs accumulate on top of
        # the Act-written mx with no PE instructions spent at all.
        # PSUM buffering: pr/pzz are single-buffered (their WAR partners rc/
        # z32 fire early in the cycle, so next-step ids never stall on them);
        # phA/phB are double-buffered so their ids don't wait on this step's
        # tanh reads. Banks: p1(2) + r(1) + z(1) + hhA(2) + hhB(2) = 8.
        def alloc_rz(stg, s, ids):
            pr = psum_p.tile([128, 4, 1, BL], F32, tag="r", name="pr", bufs=1)
            pzz = psum_z.tile([128, 4, 1, BL], F32, tag="z", name="pzz",
                              bufs=1)
            if ids:
                nc.tensor.matmul(pr[:, :, 0, :], lhsT=ident,
                                 rhs=stg[:, s, 4:8, :],
                                 start=True, stop=False, skip_group_check=True)
                nc.tensor.matmul(pzz[:, :, 0, :], lhsT=ident,
                                 rhs=stg[:, s, 0:4, :],
                                 start=True, stop=False, skip_group_check=True)
            else:
                nc.scalar.activation(out=pr[:, :, 0, :], in_=stg[:, s, 4:8, :],
                                     func=Act.Copy)
                nc.scalar.activation(out=pzz[:, :, 0, :],
                                     in_=stg[:, s, 0:4, :], func=Act.Copy)
            return pr, pzz

        def alloc_hh(stg, s, ids):
            phA = psum_1.tile([128, 2, 1, BL], F32, tag="hhA", name="phA",
                              bufs=2)
            phB = psum_1.tile([128, 2, 1, BL], F32, tag="hhB", name="phB",
                              bufs=2)
            if ids:
                nc.tensor.matmul(phA[:, :, 0, :], lhsT=ident,
                                 rhs=stg[:, s, 8:10, :],
                                 start=True, stop=False, skip_group_check=True)
                nc.tensor.matmul(phB[:, :, 0, :], lhsT=ident,
                                 rhs=stg[:, s, 10:12, :],
                                 start=True, stop=False, skip_group_check=True)
            else:
                nc.scalar.activation(out=phA[:, :, 0, :],
                                     in_=stg[:, s, 8:10, :], func=Act.Copy)
                nc.scalar.activation(out=phB[:, :, 0, :],
                                     in_=stg[:, s, 10:12, :], func=Act.Copy)
            return phA, phB

        def step(stg, s, slot, tiles, nxt, next_ids=False, extra=None):
            out_slot = (slot + 1) % BODY
            h_in = hist[:, :, slot, :]                    # [128, UC, BL] bf16
            h_in4 = hist[:, :, slot:slot + 1, :]          # [128, UC, 1, BL]
            pr, pzz, phA, phB = tiles
            # r-gate weight MMs first, k-outer so the k=0,1 MMs only need the
            # first half of the blended h (chunked handoff from prev step)
            for k in range(UC):
                for m in range(4):
                    r_i = nc.tensor.matmul(
                        pr[:, m, 0, :],
                        lhsT=wr_sb[:, k, (4 + m) * 128:(5 + m) * 128],
                        rhs=h_in[:, k, :],
                        start=False,
                        stop=(k == UC - 1 and m == 3),
                        skip_group_check=True,
                    )
            for k in range(UC):
                for m in range(4):
                    z_i = nc.tensor.matmul(
                        pzz[:, m, 0, :],
                        lhsT=wr_sb[:, k, m * 128:(m + 1) * 128],
                        rhs=h_in[:, k, :],
                        start=False,
                        stop=(k == UC - 1 and m == 3),
                        skip_group_check=True,
                    )
                    add_dep_helper(z_i.ins, r_i.ins, sync=False,
                                   reason="keep all r MMs before z MMs")

            # r32 = clip(psum_r, 0, 128) = 128*r;  rh' = r32*h = 128*(r*h)
            # (unblocks hh matmuls)
            r_bf = work.tile([128, 4, 1, BL], BF16, tag="rbf")
            nc.vector.tensor_scalar(r_bf, pr, S_GATE, 0.0,
                                    op0=Alu.min, op1=Alu.max)
            # rh in halves: the hh k=0,1 matmuls wait only on rh chunks 0,1
            # (subtile deps), starting ~110ns earlier than a full-rh op
            rh = work.tile([128, UC, 1, BL], BF16, tag="rh")
            rha_i = nc.vector.tensor_mul(rh[:, 0:2, :, :], r_bf[:, 0:2, :, :],
                                         h_in4[:, 0:2, :, :])
            rh_i = nc.vector.tensor_mul(rh[:, 2:4, :, :], r_bf[:, 2:4, :, :],
                                        h_in4[:, 2:4, :, :])
            add_dep_helper(rh_i.ins, rha_i.ins, sync=False,
                           reason="rh chunks 01 first for early hh start")
            # hh pre-activation: psum = mx_h' + rh' @ W_h'; m-halves so
            # tanh_A can run while the second-half matmuls still execute
            for m in range(2):
                for k in range(UC):
                    nc.tensor.matmul(
                        phA[:, m, 0, :],
                        lhsT=wr_sb[:, k, 2 * U + m * 128:2 * U + (m + 1) * 128],
                        rhs=rh[:, k, 0, :],
                        start=False,
                        stop=(m == 1 and k == UC - 1),
                        skip_group_check=True,
                    )
            for m in range(2, 4):
                for k in range(UC):
                    nc.tensor.matmul(
                        phB[:, m - 2, 0, :],
                        lhsT=wr_sb[:, k, 2 * U + m * 128:2 * U + (m + 1) * 128],
                        rhs=rh[:, k, 0, :],
                        start=False,
                        stop=(m == 3 and k == UC - 1),
                        skip_group_check=True,
                    )
            # z ops off the critical chain (clip_z ordered after rh):
            # z32 = clip(psum_z, 0, 128) = 128*z; he = z*h = (z32*h)/128
            # (exact: /128 is a power of two) -- he does NOT depend on w, so
            # it clears the DVE queue before the post-tanh f/add tail;
            # w = 1 - z32/128 is only needed by f.
            z32 = work.tile([128, 4, 1, BL], F32, tag="z32")
            zb_i = nc.vector.tensor_scalar(z32, pzz, S_GATE, 0.0,
                                           op0=Alu.min, op1=Alu.max)
            add_dep_helper(zb_i.ins, rh_i.ins, sync=False,
                           reason="DVE critical chain first")
            a32 = work.tile([128, 4, 1, BL], F32, tag="a32")
            nc.vector.tensor_mul(a32, z32, h_in4)                   # 128*z*h
            he_t = work.tile([128, 4, 1, BL], F32, tag="het")
            he_i = nc.vector.tensor_scalar(he_t, a32, 1.0 / S_GATE, 0.0,
                                           op0=Alu.mult, op1=Alu.bypass)
            w_t = work.tile([128, 4, 1, BL], F32, tag="wt")
            nc.vector.tensor_scalar(w_t, z32, -1.0 / S_GATE, 1.0,
                                    op0=Alu.mult, op1=Alu.add)      # 1-z
            # hh = tanh(psum/S_HH); h' = he + w*hh, in halves -> hist out_slot
            hh_A = work.tile([128, 2, 1, BL], F32, tag="hhA2")
            nc.scalar.activation(out=hh_A, in_=phA,
                                 func=Act.Tanh, scale=1.0 / S_HH)
            f_A = work.tile([128, 2, 1, BL], F32, tag="ftA")
            fa_i = nc.vector.tensor_mul(f_A, w_t[:, 0:2, :, :], hh_A)
            add_dep_helper(fa_i.ins, he_i.ins, sync=False,
                           reason="he off-critical, keep before f_A")
            # priority bump: the scheduler otherwise slots f_B between f_A
            # and add_A, delaying the next step's r matmuls by ~200ns
            with tc.high_priority(offset=6):
                adda_i = nc.vector.tensor_add(
                    hist[:, 0:2, out_slot:out_slot + 1, :],
                    f_A, he_t[:, 0:2, :, :])
            hh_B = work.tile([128, 2, 1, BL], F32, tag="hhB2")
            nc.scalar.activation(out=hh_B, in_=phB,
                                 func=Act.Tanh, scale=1.0 / S_HH)
            f_B = work.tile([128, 2, 1, BL], F32, tag="ftB")
            fb_i = nc.vector.tensor_mul(f_B, w_t[:, 2:4, :, :], hh_B)
            add_dep_helper(fb_i.ins, adda_i.ins, sync=False,
                           reason="add_A gates next r MMs, keep before f_B")
            nc.vector.tensor_add(hist[:, 2:4, out_slot:out_slot + 1, :],
                                 f_B, he_t[:, 2:4, :, :])
            # next step's PSUM mx-init ids, emitted at the tail so they run
            # back-to-back in the blend idle window: phA/phB first (bufs=2,
            # wait-free), then pr/pzz (their WAR partners rc/z32 of THIS
            # step have already fired by then)
            tiles_next = None
            if extra is not None:
                p1_pair(*extra)
            if nxt is not None:
                tiles_next = alloc_rz(*nxt, ids=next_ids) \
                    + alloc_hh(*nxt, ids=next_ids)
            return tiles_next

        # Run a 64-step sub-block: the first step's PSUM init at the block
        # top (one id group per 64 steps primes values + has_written bits),
        # every later step's init one-ahead in the previous step's tail via
        # Act copies. Keeping the carry inside the sub-block avoids
        # loop-carried psum tiles across the For_i back-edge (which
        # deadlocks the tile scheduler). prime_two=True also uses id-MMs for
        # step 1 (the very first block: that buffer set's has_written bits
        # have never been set by any matmul yet).
        def run_block(stg, base_slot, extras=None, prime_two=False):
            tiles = alloc_rz(stg, 0, ids=True) + alloc_hh(stg, 0, ids=True)
            for s in range(SBLK):
                nxt = (stg, s + 1) if s < SBLK - 1 else None
                extra = extras.get(s) if extras else None
                tiles = step(stg, s, base_slot + s, tiles, nxt,
                             next_ids=True, extra=extra)

        # ---- peeled first block (t = 0..BODY-1), python-level so the
        # remaining phase-1 t-blocks can be interleaved into its steps,
        # one piece every 3rd step (deadlines: tb1 before the stgA refill
        # at step 64, tb2/tb3 before the For_i prefetches need them) ----
        p1_sched = {2 * i: p1_rest[i] for i in range(len(p1_rest))}
        nc.sync.dma_start(out=stgB, in_=mx_sb[:, SBLK:2 * SBLK, :, :])
        run_block(stgA, 0, extras={s: p for s, p in p1_sched.items()
                                   if s < SBLK}, prime_two=True)
        nc.sync.dma_start(out=stgA, in_=mx_sb[:, BODY:BODY + SBLK, :, :])
        run_block(stgB, SBLK, extras={s - SBLK: p for s, p in p1_sched.items()
                                      if s >= SBLK})
        nc.sync.dma_start(out=out[:, :, 0:BODY - 1, :],
                          in_=hist[:, :, 1:BODY, :])
        nc.sync.dma_start(out=out[:, :, BODY - 1:BODY, :],
                          in_=hist[:, :, 0:1, :])

        with tc.For_i(BODY, T, BODY, staggered_reset=True,
                      hint_engines=(ET.PE, ET.DVE, ET.Activation,
                                    ET.SP, ET.Pool)) as i:
            nc.sync.dma_start(out=stgB,
                              in_=mx_sb[:, bass.ds(i + SBLK, SBLK), :, :])
            run_block(stgA, 0)
            nc.sync.dma_start(out=stgA,
                              in_=mx_sb[:, bass.ds(i + BODY, SBLK), :, :])
            run_block(stgB, SBLK)
            nc.sync.dma_start(out=out[:, :, bass.ds(i, BODY - 1), :],
                              in_=hist[:, :, 1:BODY, :])
            nc.sync.dma_start(out=out[:, :, bass.ds(i + BODY - 1, 1), :],
                              in_=hist[:, :, 0:1, :])
    return nc


def _graph():
    if "nc" not in _CACHE:
        nc = _build()
        if not nc.is_finalized():
            nc.finalize()
        _CACHE["nc"] = nc
    return _CACHE["nc"]


def kernel(x, kernel, recurrent_kernel, bias):
    global LAST_RESULT
    x = np.asarray(x, dtype=np.float32)
    wk_f = np.asarray(kernel, dtype=np.float32)
    wr_f = np.asarray(recurrent_kernel, dtype=np.float32)
    b_f = np.asarray(bias, dtype=np.float32)

    # fold the hard_sigmoid 0.2 and the fp8/gate scales into weights+bias
    scale = np.full((3 * U,), S_WH, np.float32)
    scale[: 2 * U] = S_WZR
    wk_scale = np.full((3 * U,), S_HH, np.float32)
    wk_scale[: 2 * U] = S_WZR
    wk_h = (wk_f * wk_scale).astype(ml_dtypes.bfloat16)
    wr_h = np.clip(wr_f * scale, -15.0, 15.0).astype(ml_dtypes.float8_e3m4)
    bp_h = np.where(np.arange(3 * U) < 2 * U,
                    S_GATE * (0.2 * b_f + 0.5), S_HH * b_f).astype(np.float32)

    in_maps = []
    for c in range(NCORES):
        xs = x[c * BL:(c + 1) * BL]                       # [BL, T, D]
        xTc = np.ascontiguousarray(
            xs.transpose(2, 1, 0).reshape(D, T * BL)      # t-major
        ).astype(ml_dtypes.bfloat16)
        in_maps.append({"xT": xTc, "wk": wk_h, "wr": wr_h, "bp": bp_h})

    res = run_bass_kernel_spmd(
        _graph(), in_maps, core_ids=list(range(NCORES)),
        trace=bool(os.environ.get("GRU_TRACE")),
    )
    LAST_RESULT = res

    outs = []
    for c in range(NCORES):
        arr = np.asarray(res.results[c]["out"]).astype(np.float32)
        outs.append(np.transpose(arr, (3, 2, 1, 0)).reshape(BL, T, U))
    return np.concatenate(outs, axis=0)
